# revision 38
# baseline (speedup 1.0000x reference)
"""Trainium2 Bass kernel for nn_DiWeightedGCNLayer (8-core SPMD), v2.

Math (per reference):
    h   = LayerNorm(x) * gamma + beta        (beta=0, b=0 here)
    m   = h @ W.T + b
    msg = m[src] * w
    out = segment_sum(msg, dst) / max(segment_sum(w, dst), 1) * dst_scale
    y   = x + gelu(out)

v2 design (vs the indirect-DMA baseline):
  Phase 1 (sharded): each core computes m for its 1/8 of node rows.
    LN's mean-subtraction is folded into the weight matrix on the host
    (W2c = W2 - ones @ colsum(W2)/D), so per 128-row tile we do:
    PE-transpose(x_bf16) -> matmul(x_T, W2c) -> scale rows by
    rstd = rsqrt(var+eps) (Act engine) -> m_part (bf16, HBM).
    An AllGather collective assembles the full m on every core.
  Phase 2: edges sorted by (dst chunk, src-half). Messages are fetched
    with batched gpsimd.dma_gather (hundreds of 256B rows per call,
    994ns fixed cost amortized) instead of one indirect DMA per 128
    edges. Scatter-add per 128-dst-node chunk stays the one-hot PE
    matmul (oh = (iota==rel)*w built by DVE).
    int16 gather indices force a split of m rows at 32768: each chunk's
    edges are grouped into src<32768 and src>=32768 blocks, gathered by
    two calls with different base row offsets.
"""

import contextlib
import numpy as np
import ml_dtypes

import concourse.bass as bass
import concourse.bacc as bacc
import concourse.tile as tile
import concourse.mybir as mybir
from concourse.bass_utils import run_bass_kernel_spmd

F32 = mybir.dt.float32
BF16 = mybir.dt.bfloat16
I32 = mybir.dt.int32
I16 = mybir.dt.int16
AF = mybir.ActivationFunctionType
OP = mybir.AluOpType

D = 128
P = 128
LN_EPS = 1e-5
R = 8
HALF = 32768
GC = 4  # chunks per gather group
SHARD_P1 = True  # ship sharded phase 1 + AllGather


def layout_blocks(bh, gc=GC):
    """Global block-column layout: group-major, half-major inside a group.
    Returns (TB, sbo, call_specs, group_spans):
      sbo[ci] = (col of ci's first h0 block, col of ci's first h1 block)
      call_specs[g] = (tb0, nb0, tb1, nb1)  (h0/h1 stream offsets+lengths)
      group_spans[g] = (ci_start, ci_end, tb_base, nb_total)
    """
    nch = len(bh)
    tb = 0
    sbo = {}
    call_specs = []
    group_spans = []
    for cs in range(0, nch, gc):
        g = list(range(cs, min(nch, cs + gc)))
        nb0 = sum(bh[ci][0] for ci in g)
        nb1 = sum(bh[ci][1] for ci in g)
        cur0, cur1 = tb, tb + nb0
        for ci in g:
            sbo[ci] = (cur0, cur1)
            cur0 += bh[ci][0]
            cur1 += bh[ci][1]
        call_specs.append((tb, nb0, tb + nb0, nb1))
        group_spans.append((cs, min(nch, cs + gc), tb, nb0 + nb1))
        tb += nb0 + nb1
    return tb, sbo, call_specs, group_spans


def build_program(n_pad2, nch, bh, shard_p1=True, loop_n=1, g_tiles=None,
                  n_swdge=4, msg_bufs=6, oh_bufs=8, skip_gather=False,
                  skip_p1=False, cc_emu=False, shared_m=False,
                  skip_compute=False, skip_mm=False, oh_pool_frac=0.0):
    """One-core SPMD program. bh: tuple of (h0_blocks, h1_blocks) per chunk
    (identical across cores = max over cores)."""
    rows_pc = n_pad2 // R          # node rows computed per core in phase 1
    tpc = rows_pc // P             # tiles per core
    nt_p1 = tpc if shard_p1 else n_pad2 // P
    if g_tiles is None:
        g_tiles = 7 if shard_p1 else 14
    assert nt_p1 % g_tiles == 0
    TB, sbo, call_specs, group_spans = layout_blocks(bh)

    nc = bacc.Bacc(num_swdge_queues=n_swdge, num_devices=R)

    xp_rows = rows_pc if shard_p1 else n_pad2
    xp_ext = nc.declare_dram_parameter("xp", [xp_rows, D], BF16, isOutput=False)
    xres_ext = nc.declare_dram_parameter("xres", [nch * P, D], F32,
                                         isOutput=False)
    w2c_ext = nc.declare_dram_parameter("w2c", [D, D], BF16, isOutput=False)
    iota_ext = nc.declare_dram_parameter("iota", [P, P], BF16, isOutput=False)
    ident_ext = nc.declare_dram_parameter("ident", [P, P], BF16, isOutput=False)
    idx_ext = nc.declare_dram_parameter("gidx", [P, TB * 8], I16, isOutput=False)
    rel_ext = nc.declare_dram_parameter("rels", [P, TB], F32, isOutput=False)
    w_ext = nc.declare_dram_parameter("ws", [P, TB], F32, isOutput=False)
    dsc_ext = nc.declare_dram_parameter("dsct", [P, nch], F32, isOutput=False)
    y_ext = nc.declare_dram_parameter("y", [nch * P, D], F32, isOutput=True)

    m_full = nc.dram_tensor("m_full", [n_pad2, D], BF16,
                            addr_space="Shared" if shared_m else "Local")
    if shard_p1:
        m_part = nc.dram_tensor("m_part", [rows_pc, D], BF16)
    if cc_emu and not shared_m:
        mrest_ext = nc.declare_dram_parameter(
            "m_rest", [n_pad2 - rows_pc, D], BF16, isOutput=False)

    with tile.TileContext(nc) as tc:
        with (
            tc.tile_pool(name="const", bufs=1) as const,
            tc.tile_pool(name="xp", bufs=3) as xpp,
            tc.tile_pool(name="stats", bufs=4) as sp,
            tc.tile_pool(name="small", bufs=6) as smp,
            tc.tile_pool(name="xts", bufs=3) as xtsp,
            tc.tile_pool(name="mp", bufs=3) as mp,
            tc.tile_pool(name="msg", bufs=msg_bufs) as msgp,
            tc.tile_pool(name="oh", bufs=oh_bufs) as ohp,
            tc.tile_pool(name="ep", bufs=4) as epp,
            tc.tile_pool(name="ps_t", bufs=2, space="PSUM") as ps_t,
            tc.tile_pool(name="ps_m", bufs=2, space="PSUM") as ps_m,
            tc.tile_pool(name="ps_o", bufs=3, space="PSUM") as ps_o,
        ):
            # --- constants (outside the benchmark loop) ---
            w2c_t = const.tile([D, D], BF16)
            nc.sync.dma_start(out=w2c_t[:], in_=w2c_ext[:, :])
            iota_t = const.tile([P, P], BF16)
            nc.sync.dma_start(out=iota_t[:], in_=iota_ext[:, :])
            ident = const.tile([P, P], BF16)
            nc.sync.dma_start(out=ident[:], in_=ident_ext[:, :])
            eps_t = const.tile([P, 1], F32)
            nc.vector.memset(eps_t[:], LN_EPS)
            dsc_t = const.tile([P, nch], F32)
            nc.sync.dma_start(out=dsc_t[:], in_=dsc_ext[:, :])
            idx_t = const.tile([P, TB * 8], I16)
            nc.sync.dma_start(out=idx_t[:], in_=idx_ext[:, :])
            rel_t = const.tile([P, TB], F32)
            nc.sync.dma_start(out=rel_t[:], in_=rel_ext[:, :])
            w_t = const.tile([P, TB], F32)
            nc.sync.dma_start(out=w_t[:], in_=w_ext[:, :])

            loop_ctx = (tc.For_i(0, loop_n, 1) if loop_n > 1
                        else contextlib.nullcontext())
            with loop_ctx:
                # --- phase 1: m = rstd * (x_bf16 @ W2c), bf16 to HBM ---
                # Node rows permuted inside each supertile of 128*G rows:
                # row (t, p, j) = t*128G + p*G + j -> partition p, slot j.
                # One contiguous G*256B descriptor per partition per DMA.
                m_dst_dram = m_part if shard_p1 else m_full
                G = g_tiles
                for t0 in range(0, 0 if skip_p1 else nt_p1, G):
                    g_n = min(G, nt_p1 - t0)
                    xt4 = xpp.tile([P, G, D], BF16)
                    x_src = xp_ext[t0 * P:(t0 + g_n) * P, :].rearrange(
                        "(p j) d -> p j d", p=P)
                    nc.sync.dma_start(out=xt4[:, :g_n, :], in_=x_src)
                    m4 = mp.tile([P, G, D], BF16)
                    for j in range(g_n):
                        xt = xt4[:, j, :]
                        st = sp.tile([P, 6], F32)
                        nc.vector.bn_stats(out=st[:], in_=xt)
                        mv = sp.tile([P, 2], F32)
                        nc.vector.bn_aggr(out=mv[:], in_=st[:])
                        sd = smp.tile([P, 1], F32)
                        nc.scalar.activation(out=sd[:], in_=mv[:, 1:2],
                                             func=AF.Sqrt, bias=eps_t[:, :],
                                             scale=1.0)
                        rstd = smp.tile([P, 1], F32)
                        nc.vector.reciprocal(out=rstd[:], in_=sd[:])
                        xt_ps = ps_t.tile([P, D], BF16)
                        nc.tensor.transpose(out=xt_ps[:], in_=xt,
                                            identity=ident[:])
                        xts = xtsp.tile([P, D], BF16)
                        nc.scalar.copy(out=xts[:], in_=xt_ps[:])
                        m_ps = ps_m.tile([P, D], F32)
                        nc.tensor.matmul(out=m_ps[:], lhsT=xts[:], rhs=w2c_t[:],
                                         start=True, stop=True)
                        nc.vector.tensor_scalar(out=m4[:, j, :], in0=m_ps[:],
                                                scalar1=rstd[:], scalar2=None,
                                                op0=OP.mult)
                    m_dst = m_dst_dram[t0 * P:(t0 + g_n) * P, :].rearrange(
                        "(p j) d -> p j d", p=P)
                    nc.sync.dma_start(out=m_dst, in_=m4[:, :g_n, :])

                if shard_p1 and cc_emu:
                    # benchmark stand-in for AllGather (loop-unsafe on HW):
                    # local HBM copies with the same written byte count
                    nc.sync.dma_start(out=m_full[0:rows_pc, :],
                                      in_=m_part[:, :])
                    if not shared_m:
                        nc.sync.dma_start(out=m_full[rows_pc:n_pad2, :],
                                          in_=mrest_ext[:, :])
                elif shard_p1:
                    nc.gpsimd.collective_compute(
                        "AllGather", OP.bypass,
                        replica_groups=[list(range(R))],
                        ins=[m_part[:, :]],
                        outs=[m_full[:, :]],
                    )

                # --- phase 2: batched gather + one-hot scatter matmul ---
                qi = 0
                nreg_cache = {}
                for gidx, (cs, ce, tb_base, nb_tot) in enumerate(group_spans):
                    tb0, nb0, tb1, nb1 = call_specs[gidx]
                    msg = msgp.tile([P, nb_tot, D], BF16)
                    # SWDGE ring caps one call at 1024 descriptors (8 blocks)
                    CB = 8
                    for h, (tbh, nbh) in enumerate(((tb0, nb0), (tb1, nb1))):
                        base = h * HALF
                        for s0 in range(0, nbh, CB):
                            sn = min(CB, nbh - s0)
                            L = sn * P
                            col0 = tbh - tb_base + s0
                            if skip_gather:
                                nc.vector.memset(msg[:, col0:col0 + sn, :],
                                                 0.25)
                                continue
                            if L not in nreg_cache:
                                nreg_cache[L] = nc.gpsimd.to_reg(L)
                            nc.gpsimd.dma_gather(
                                msg[:, col0:col0 + sn, :],
                                m_full[base:n_pad2, :],
                                idx_t[:, (tbh + s0) * 8:(tbh + s0 + sn) * 8],
                                num_idxs=L,
                                num_idxs_reg=nreg_cache[L],
                                elem_size=D,
                                queue_num=qi,
                            )
                            qi = (qi + 1) % n_swdge
                    ng = ce - cs
                    xr = epp.tile([P, GC, D], F32, tag="xr")
                    xr_src = xres_ext[cs * P:ce * P, :].rearrange(
                        "(j p) d -> p j d", p=P)
                    nc.sync.dma_start(out=xr[:, :ng, :], in_=xr_src)
                    yt = epp.tile([P, GC, D], F32, tag="yt")
                    if skip_compute:
                        nc.vector.tensor_scalar(out=yt[:, 0, :],
                                                in0=msg[:, 0, :],
                                                scalar1=1.0, scalar2=None,
                                                op0=OP.mult)
                        y_dst = y_ext[cs * P:ce * P, :].rearrange(
                            "(j p) d -> p j d", p=P)
                        nc.sync.dma_start(out=y_dst, in_=yt[:, :ng, :])
                        continue
                    for ci in range(cs, ce):
                        c0, c1 = sbo[ci]
                        cols = ([c0 + b for b in range(bh[ci][0])]
                                + [c1 + b for b in range(bh[ci][1])])
                        out_ps = ps_o.tile([P, D], F32)
                        nb = len(cols)
                        for k, tb in enumerate(cols):
                            oh = ohp.tile([P, P], BF16)
                            oh_eng = (nc.gpsimd if (k % 100) < oh_pool_frac * 100
                                      else nc.vector)
                            oh_eng.tensor_scalar(out=oh[:], in0=iota_t[:],
                                                 scalar1=rel_t[:, tb:tb + 1],
                                                 scalar2=w_t[:, tb:tb + 1],
                                                 op0=OP.is_equal,
                                                 op1=OP.mult)
                            if skip_mm:
                                continue
                            mcol = tb - tb_base
                            nc.tensor.matmul(out=out_ps[:], lhsT=oh[:],
                                             rhs=msg[:, mcol, :],
                                             start=(k == 0), stop=(k == nb - 1))

                        # dsc_t holds host-precomputed dst_scale/max(deg,1)
                        sc = epp.tile([P, D], F32, tag="sc")
                        nc.vector.tensor_scalar(out=sc[:], in0=out_ps[:],
                                                scalar1=dsc_t[:, ci:ci + 1],
                                                scalar2=None, op0=OP.mult)
                        g = epp.tile([P, D], F32, tag="g")
                        nc.scalar.activation(out=g[:], in_=sc[:], func=AF.Gelu)
                        nc.vector.tensor_add(out=yt[:, ci - cs, :], in0=g[:],
                                             in1=xr[:, ci - cs, :])
                    y_dst = y_ext[cs * P:ce * P, :].rearrange(
                        "(j p) d -> p j d", p=P)
                    nc.sync.dma_start(out=y_dst, in_=yt[:, :ng, :])

    return nc


def prepare_inputs(x, gamma, beta, W, b, edge_index, edge_weight, dst_scale,
                   n_cores, shard_p1=None):
    if shard_p1 is None:
        shard_p1 = SHARD_P1
    """Host-side prep: sort edges by (dst-chunk, src-half), build gather
    index/rel/weight streams; fold LN gamma + mean-subtraction into W2c."""
    N = x.shape[0]
    assert n_cores == R
    npc = N // R                       # dst ownership per core (6250)
    nch = (npc + P - 1) // P           # 49
    n_pad2 = ((N + R * P - 1) // (R * P)) * R * P  # 50176
    rows_pc = n_pad2 // R              # 6272

    src = np.ascontiguousarray(edge_index[0]).astype(np.int64)
    dst = np.ascontiguousarray(edge_index[1]).astype(np.int64)
    w = edge_weight.astype(np.float32)
    E = src.shape[0]

    deg = np.bincount(dst, weights=w.astype(np.float64), minlength=N)
    indsc = (dst_scale.astype(np.float64)
             / np.maximum(deg, 1.0)).astype(np.float32)

    core_id = np.minimum(dst // npc, R - 1)
    local = dst - core_id * npc
    chunk_id = local // P
    rel = (local - chunk_id * P).astype(np.float32)
    half = (src >= HALF).astype(np.int64)
    key = (core_id * nch + chunk_id) * 2 + half
    order = np.argsort(key, kind="stable")
    key_s = key[order]
    src_s, rel_s, w_s = src[order], rel[order], w[order]

    cnt = np.bincount(key_s, minlength=R * nch * 2).reshape(R, nch, 2)
    bh_arr = -(-cnt.max(axis=0) // P)          # [nch, 2] blocks
    bh_arr[:, 0] = np.maximum(bh_arr[:, 0], 1)
    bh = tuple((int(a), int(b)) for a, b in bh_arr)

    TB, sbo, call_specs, group_spans = layout_blocks(bh)

    # column offset (in the 128-edge-wide stream) for each (chunk, half)
    colbase = np.zeros((nch, 2), np.int64)
    for ci in range(nch):
        colbase[ci, 0] = sbo[ci][0]
        colbase[ci, 1] = sbo[ci][1]

    starts = np.searchsorted(key_s, np.arange(R * nch * 2 + 1))
    pos = np.arange(E) - starts[key_s]
    ch_s = (key_s // 2) % nch
    hf_s = key_s % 2
    co_s = key_s // (2 * nch)
    col = colbase[ch_s, hf_s] * P + pos         # position in per-core stream

    L_stream = TB * P
    idxs = np.zeros((R, L_stream), np.int32)    # gather idx (half-relative)
    rels = np.zeros((R, L_stream), np.float32)
    ws = np.zeros((R, L_stream), np.float32)
    idxs[co_s, col] = src_s - hf_s * HALF
    rels[co_s, col] = rel_s
    ws[co_s, col] = w_s
    assert idxs.max() < HALF and idxs.min() >= 0

    # gather idx wrapping: idx i -> partition i%16, column i//16, replicated
    # to all 8 16-partition groups
    idx16 = idxs.reshape(R, TB * 8, 16).transpose(0, 2, 1)   # [R, 16, TB*8]
    idx_wrap = np.ascontiguousarray(
        np.tile(idx16, (1, 8, 1)).astype(np.int16))          # [R, 128, TB*8]
    # rel/w: edge (tb, p) -> [128, TB]
    relsT = np.ascontiguousarray(
        rels.reshape(R, TB, P).transpose(0, 2, 1))
    wsT = np.ascontiguousarray(ws.reshape(R, TB, P).transpose(0, 2, 1))

    x_pad = np.zeros((n_pad2, D), np.float32)
    x_pad[:N] = x.astype(np.float32)
    x_bf = x_pad.astype(ml_dtypes.bfloat16)

    W2 = (W.T.astype(np.float32) * gamma.astype(np.float32)[:, None])
    W2c = W2 - np.ones((D, 1), np.float32) @ (W2.sum(axis=0, keepdims=True)) / D
    W2c = W2c.astype(ml_dtypes.bfloat16)
    c = beta.astype(np.float32) @ W.T.astype(np.float32) + b.astype(np.float32)
    assert not np.any(c != 0.0), "nonzero LN beta / linear bias unsupported"

    iota = np.broadcast_to(np.arange(P, dtype=np.float32), (P, P))
    iota = np.ascontiguousarray(iota).astype(ml_dtypes.bfloat16)
    ident = np.eye(P, dtype=np.float32).astype(ml_dtypes.bfloat16)

    in_maps = []
    for r in range(R):
        lo = r * npc
        hi = min(N, lo + npc)
        dsr = np.zeros(nch * P, np.float32)
        dsr[:hi - lo] = indsc[lo:hi]
        dsct = np.ascontiguousarray(dsr.reshape(nch, P).T)
        xres = np.zeros((nch * P, D), np.float32)
        xres[:hi - lo] = x_pad[lo:hi]
        in_maps.append({
            "xp": (np.ascontiguousarray(x_bf[r * rows_pc:(r + 1) * rows_pc])
                   if shard_p1 else x_bf),
            "xres": xres,
            "w2c": W2c,
            "iota": iota,
            "ident": ident,
            "gidx": idx_wrap[r],
            "rels": relsT[r],
            "ws": wsT[r],
            "dsct": dsct,
        })
    geom = dict(n_pad2=n_pad2, nch=nch, bh=bh, npc=npc, N=N, R=R, TB=TB)
    return in_maps, geom


_PROGRAM_CACHE = {}


def kernel(x, gamma, beta, W, b, edge_index, num_nodes, edge_weight,
           dst_scale, n_cores=8, _collect=None):
    x = np.asarray(x)
    N = x.shape[0]
    in_maps, geom = prepare_inputs(
        np.asarray(x), np.asarray(gamma), np.asarray(beta), np.asarray(W),
        np.asarray(b), np.asarray(edge_index), np.asarray(edge_weight),
        np.asarray(dst_scale), n_cores)

    key = (geom["n_pad2"], geom["nch"], geom["bh"], SHARD_P1)
    nc = _PROGRAM_CACHE.get(key)
    if nc is None:
        nc = build_program(geom["n_pad2"], geom["nch"], geom["bh"],
                           shard_p1=SHARD_P1, shared_m=SHARD_P1)
        nc.finalize()
        _PROGRAM_CACHE[key] = nc

    res = run_bass_kernel_spmd(nc, in_maps, list(range(n_cores)),
                               **(_collect.pop("kwargs") if _collect else {}))
    if _collect is not None:
        _collect["res"] = res

    y = np.empty((N, D), np.float32)
    npc = geom["npc"]
    for r in range(geom["R"]):
        lo = r * npc
        hi = min(N, lo + npc)
        y[lo:hi] = res.results[r]["y"][:hi - lo]
    return y


# revision 40
# speedup vs baseline: 4.1083x; 4.1083x over previous
"""Trainium2 Bass kernel for nn_DiWeightedGCNLayer (8-core SPMD), v2.

Math (per reference):
    h   = LayerNorm(x) * gamma + beta        (beta=0, b=0 here)
    m   = h @ W.T + b
    msg = m[src] * w
    out = segment_sum(msg, dst) / max(segment_sum(w, dst), 1) * dst_scale
    y   = x + gelu(out)

v2 design (vs the indirect-DMA baseline):
  Phase 1 (sharded): each core computes m for its 1/8 of node rows.
    LN's mean-subtraction is folded into the weight matrix on the host
    (W2c = W2 - ones @ colsum(W2)/D), so per 128-row tile we do:
    PE-transpose(x_bf16) -> matmul(x_T, W2c) -> scale rows by
    rstd = rsqrt(var+eps) (Act engine) -> m_part (bf16, HBM).
    An AllGather collective assembles the full m on every core.
  Phase 2: edges sorted by (dst chunk, src-half). Messages are fetched
    with batched gpsimd.dma_gather (hundreds of 256B rows per call,
    994ns fixed cost amortized) instead of one indirect DMA per 128
    edges. Scatter-add per 128-dst-node chunk stays the one-hot PE
    matmul (oh = (iota==rel)*w built by DVE).
    int16 gather indices force a split of m rows at 32768: each chunk's
    edges are grouped into src<32768 and src>=32768 blocks, gathered by
    two calls with different base row offsets.
"""

import contextlib
import numpy as np
import ml_dtypes

import concourse.bass as bass
import concourse.bacc as bacc
import concourse.tile as tile
import concourse.mybir as mybir
from concourse.bass_utils import run_bass_kernel_spmd

F32 = mybir.dt.float32
BF16 = mybir.dt.bfloat16
I32 = mybir.dt.int32
I16 = mybir.dt.int16
AF = mybir.ActivationFunctionType
OP = mybir.AluOpType

D = 128
P = 128
LN_EPS = 1e-5
R = 8
HALF = 32768
GC = 4  # chunks per gather group
SHARD_P1 = True  # ship sharded phase 1 + AllGather


def layout_blocks(bh, gc=GC):
    """Global block-column layout: group-major, half-major inside a group.
    Returns (TB, sbo, call_specs, group_spans):
      sbo[ci] = (col of ci's first h0 block, col of ci's first h1 block)
      call_specs[g] = (tb0, nb0, tb1, nb1)  (h0/h1 stream offsets+lengths)
      group_spans[g] = (ci_start, ci_end, tb_base, nb_total)
    """
    nch = len(bh)
    tb = 0
    sbo = {}
    call_specs = []
    group_spans = []
    for cs in range(0, nch, gc):
        g = list(range(cs, min(nch, cs + gc)))
        nb0 = sum(bh[ci][0] for ci in g)
        nb1 = sum(bh[ci][1] for ci in g)
        cur0, cur1 = tb, tb + nb0
        for ci in g:
            sbo[ci] = (cur0, cur1)
            cur0 += bh[ci][0]
            cur1 += bh[ci][1]
        call_specs.append((tb, nb0, tb + nb0, nb1))
        group_spans.append((cs, min(nch, cs + gc), tb, nb0 + nb1))
        tb += nb0 + nb1
    return tb, sbo, call_specs, group_spans


def build_program(n_pad2, nch, bh, shard_p1=True, loop_n=1, g_tiles=None,
                  n_swdge=4, msg_bufs=6, oh_bufs=8, skip_gather=False,
                  skip_p1=False, cc_emu=False, shared_m=False,
                  skip_compute=False, skip_mm=False, oh_pool_frac=0.0,
                  unroll=False):
    """One-core SPMD program. bh: tuple of (h0_blocks, h1_blocks) per chunk
    (identical across cores = max over cores)."""
    rows_pc = n_pad2 // R          # node rows computed per core in phase 1
    tpc = rows_pc // P             # tiles per core
    nt_p1 = tpc if shard_p1 else n_pad2 // P
    if g_tiles is None:
        g_tiles = 7 if shard_p1 else 14
    assert nt_p1 % g_tiles == 0
    TB, sbo, call_specs, group_spans = layout_blocks(bh)

    nc = bacc.Bacc(num_swdge_queues=n_swdge, num_devices=R)

    xp_rows = rows_pc if shard_p1 else n_pad2
    xp_ext = nc.declare_dram_parameter("xp", [xp_rows, D], BF16, isOutput=False)
    xres_ext = nc.declare_dram_parameter("xres", [nch * P, D], F32,
                                         isOutput=False)
    w2c_ext = nc.declare_dram_parameter("w2c", [D, D], BF16, isOutput=False)
    iota_ext = nc.declare_dram_parameter("iota", [P, P], BF16, isOutput=False)
    ident_ext = nc.declare_dram_parameter("ident", [P, P], BF16, isOutput=False)
    idx_ext = nc.declare_dram_parameter("gidx", [P, TB * 8], I16, isOutput=False)
    rel_ext = nc.declare_dram_parameter("rels", [P, TB], F32, isOutput=False)
    w_ext = nc.declare_dram_parameter("ws", [P, TB], F32, isOutput=False)
    dsc_ext = nc.declare_dram_parameter("dsct", [P, nch], F32, isOutput=False)
    y_ext = nc.declare_dram_parameter("y", [nch * P, D], F32, isOutput=True)

    m_full = nc.dram_tensor("m_full", [n_pad2, D], BF16,
                            addr_space="Shared" if shared_m else "Local")
    if shard_p1:
        m_part = nc.dram_tensor("m_part", [rows_pc, D], BF16)
    if cc_emu and not shared_m:
        mrest_ext = nc.declare_dram_parameter(
            "m_rest", [n_pad2 - rows_pc, D], BF16, isOutput=False)

    with tile.TileContext(nc) as tc:
        with (
            tc.tile_pool(name="const", bufs=1) as const,
            tc.tile_pool(name="xp", bufs=3) as xpp,
            tc.tile_pool(name="stats", bufs=4) as sp,
            tc.tile_pool(name="small", bufs=6) as smp,
            tc.tile_pool(name="xts", bufs=3) as xtsp,
            tc.tile_pool(name="mp", bufs=3) as mp,
            tc.tile_pool(name="msg", bufs=msg_bufs) as msgp,
            tc.tile_pool(name="oh", bufs=oh_bufs) as ohp,
            tc.tile_pool(name="ep", bufs=4) as epp,
            tc.tile_pool(name="ps_t", bufs=2, space="PSUM") as ps_t,
            tc.tile_pool(name="ps_m", bufs=2, space="PSUM") as ps_m,
            tc.tile_pool(name="ps_o", bufs=3, space="PSUM") as ps_o,
        ):
            # --- constants (outside the benchmark loop) ---
            w2c_t = const.tile([D, D], BF16)
            nc.sync.dma_start(out=w2c_t[:], in_=w2c_ext[:, :])
            iota_t = const.tile([P, P], BF16)
            nc.sync.dma_start(out=iota_t[:], in_=iota_ext[:, :])
            ident = const.tile([P, P], BF16)
            nc.sync.dma_start(out=ident[:], in_=ident_ext[:, :])
            eps_t = const.tile([P, 1], F32)
            nc.vector.memset(eps_t[:], LN_EPS)
            dsc_t = const.tile([P, nch], F32)
            nc.sync.dma_start(out=dsc_t[:], in_=dsc_ext[:, :])
            idx_t = const.tile([P, TB * 8], I16)
            nc.sync.dma_start(out=idx_t[:], in_=idx_ext[:, :])
            rel_t = const.tile([P, TB], F32)
            nc.sync.dma_start(out=rel_t[:], in_=rel_ext[:, :])
            w_t = const.tile([P, TB], F32)
            nc.sync.dma_start(out=w_t[:], in_=w_ext[:, :])

            loop_ctx = (tc.For_i(0, loop_n, 1) if loop_n > 1 and not unroll
                        else contextlib.nullcontext())
            for _unroll_i in range(loop_n if unroll else 1):
              with loop_ctx:
                # --- phase 1: m = rstd * (x_bf16 @ W2c), bf16 to HBM ---
                # Node rows permuted inside each supertile of 128*G rows:
                # row (t, p, j) = t*128G + p*G + j -> partition p, slot j.
                # One contiguous G*256B descriptor per partition per DMA.
                m_dst_dram = m_part if shard_p1 else m_full
                G = g_tiles
                for t0 in range(0, 0 if skip_p1 else nt_p1, G):
                    g_n = min(G, nt_p1 - t0)
                    xt4 = xpp.tile([P, G, D], BF16)
                    x_src = xp_ext[t0 * P:(t0 + g_n) * P, :].rearrange(
                        "(p j) d -> p j d", p=P)
                    nc.sync.dma_start(out=xt4[:, :g_n, :], in_=x_src)
                    m4 = mp.tile([P, G, D], BF16)
                    for j in range(g_n):
                        xt = xt4[:, j, :]
                        st = sp.tile([P, 6], F32)
                        nc.vector.bn_stats(out=st[:], in_=xt)
                        mv = sp.tile([P, 2], F32)
                        nc.vector.bn_aggr(out=mv[:], in_=st[:])
                        sd = smp.tile([P, 1], F32)
                        nc.scalar.activation(out=sd[:], in_=mv[:, 1:2],
                                             func=AF.Sqrt, bias=eps_t[:, :],
                                             scale=1.0)
                        rstd = smp.tile([P, 1], F32)
                        nc.vector.reciprocal(out=rstd[:], in_=sd[:])
                        xt_ps = ps_t.tile([P, D], BF16)
                        nc.tensor.transpose(out=xt_ps[:], in_=xt,
                                            identity=ident[:])
                        xts = xtsp.tile([P, D], BF16)
                        nc.scalar.copy(out=xts[:], in_=xt_ps[:])
                        m_ps = ps_m.tile([P, D], F32)
                        nc.tensor.matmul(out=m_ps[:], lhsT=xts[:], rhs=w2c_t[:],
                                         start=True, stop=True)
                        nc.vector.tensor_scalar(out=m4[:, j, :], in0=m_ps[:],
                                                scalar1=rstd[:], scalar2=None,
                                                op0=OP.mult)
                    m_dst = m_dst_dram[t0 * P:(t0 + g_n) * P, :].rearrange(
                        "(p j) d -> p j d", p=P)
                    nc.sync.dma_start(out=m_dst, in_=m4[:, :g_n, :])

                if shard_p1 and cc_emu:
                    # benchmark stand-in for AllGather (loop-unsafe on HW):
                    # local HBM copies with the same written byte count
                    nc.sync.dma_start(out=m_full[0:rows_pc, :],
                                      in_=m_part[:, :])
                    if not shared_m:
                        nc.sync.dma_start(out=m_full[rows_pc:n_pad2, :],
                                          in_=mrest_ext[:, :])
                elif shard_p1:
                    nc.gpsimd.collective_compute(
                        "AllGather", OP.bypass,
                        replica_groups=[list(range(R))],
                        ins=[m_part[:, :]],
                        outs=[m_full[:, :]],
                    )

                # --- phase 2: batched gather + one-hot scatter matmul ---
                qi = 0
                nreg_cache = {}
                for gidx, (cs, ce, tb_base, nb_tot) in enumerate(group_spans):
                    tb0, nb0, tb1, nb1 = call_specs[gidx]
                    msg = msgp.tile([P, nb_tot, D], BF16)
                    # SWDGE ring caps one call at 1024 descriptors (8 blocks)
                    CB = 8
                    for h, (tbh, nbh) in enumerate(((tb0, nb0), (tb1, nb1))):
                        base = h * HALF
                        for s0 in range(0, nbh, CB):
                            sn = min(CB, nbh - s0)
                            L = sn * P
                            col0 = tbh - tb_base + s0
                            if skip_gather:
                                nc.vector.memset(msg[:, col0:col0 + sn, :],
                                                 0.25)
                                continue
                            if L not in nreg_cache:
                                nreg_cache[L] = nc.gpsimd.to_reg(L)
                            nc.gpsimd.dma_gather(
                                msg[:, col0:col0 + sn, :],
                                m_full[base:n_pad2, :],
                                idx_t[:, (tbh + s0) * 8:(tbh + s0 + sn) * 8],
                                num_idxs=L,
                                num_idxs_reg=nreg_cache[L],
                                elem_size=D,
                                queue_num=qi,
                            )
                            qi = (qi + 1) % n_swdge
                    ng = ce - cs
                    xr = epp.tile([P, GC, D], F32, tag="xr")
                    xr_src = xres_ext[cs * P:ce * P, :].rearrange(
                        "(j p) d -> p j d", p=P)
                    nc.sync.dma_start(out=xr[:, :ng, :], in_=xr_src)
                    yt = epp.tile([P, GC, D], F32, tag="yt")
                    if skip_compute:
                        nc.vector.tensor_scalar(out=yt[:, 0, :],
                                                in0=msg[:, 0, :],
                                                scalar1=1.0, scalar2=None,
                                                op0=OP.mult)
                        y_dst = y_ext[cs * P:ce * P, :].rearrange(
                            "(j p) d -> p j d", p=P)
                        nc.sync.dma_start(out=y_dst, in_=yt[:, :ng, :])
                        continue
                    for ci in range(cs, ce):
                        c0, c1 = sbo[ci]
                        cols = ([c0 + b for b in range(bh[ci][0])]
                                + [c1 + b for b in range(bh[ci][1])])
                        out_ps = ps_o.tile([P, D], F32)
                        nb = len(cols)
                        for k, tb in enumerate(cols):
                            oh = ohp.tile([P, P], BF16)
                            oh_eng = (nc.gpsimd if (k % 100) < oh_pool_frac * 100
                                      else nc.vector)
                            oh_eng.tensor_scalar(out=oh[:], in0=iota_t[:],
                                                 scalar1=rel_t[:, tb:tb + 1],
                                                 scalar2=w_t[:, tb:tb + 1],
                                                 op0=OP.is_equal,
                                                 op1=OP.mult)
                            if skip_mm:
                                continue
                            mcol = tb - tb_base
                            nc.tensor.matmul(out=out_ps[:], lhsT=oh[:],
                                             rhs=msg[:, mcol, :],
                                             start=(k == 0), stop=(k == nb - 1))

                        # dsc_t holds host-precomputed dst_scale/max(deg,1)
                        sc = epp.tile([P, D], F32, tag="sc")
                        nc.vector.tensor_scalar(out=sc[:], in0=out_ps[:],
                                                scalar1=dsc_t[:, ci:ci + 1],
                                                scalar2=None, op0=OP.mult)
                        g = epp.tile([P, D], F32, tag="g")
                        nc.scalar.activation(out=g[:], in_=sc[:], func=AF.Gelu)
                        nc.vector.tensor_add(out=yt[:, ci - cs, :], in0=g[:],
                                             in1=xr[:, ci - cs, :])
                    y_dst = y_ext[cs * P:ce * P, :].rearrange(
                        "(j p) d -> p j d", p=P)
                    nc.sync.dma_start(out=y_dst, in_=yt[:, :ng, :])

    return nc


def prepare_inputs(x, gamma, beta, W, b, edge_index, edge_weight, dst_scale,
                   n_cores, shard_p1=None):
    if shard_p1 is None:
        shard_p1 = SHARD_P1
    """Host-side prep: sort edges by (dst-chunk, src-half), build gather
    index/rel/weight streams; fold LN gamma + mean-subtraction into W2c."""
    N = x.shape[0]
    assert n_cores == R
    npc = N // R                       # dst ownership per core (6250)
    nch = (npc + P - 1) // P           # 49
    n_pad2 = ((N + R * P - 1) // (R * P)) * R * P  # 50176
    rows_pc = n_pad2 // R              # 6272

    src = np.ascontiguousarray(edge_index[0]).astype(np.int64)
    dst = np.ascontiguousarray(edge_index[1]).astype(np.int64)
    w = edge_weight.astype(np.float32)
    E = src.shape[0]

    deg = np.bincount(dst, weights=w.astype(np.float64), minlength=N)
    indsc = (dst_scale.astype(np.float64)
             / np.maximum(deg, 1.0)).astype(np.float32)

    core_id = np.minimum(dst // npc, R - 1)
    local = dst - core_id * npc
    chunk_id = local // P
    rel = (local - chunk_id * P).astype(np.float32)
    half = (src >= HALF).astype(np.int64)
    key = (core_id * nch + chunk_id) * 2 + half
    order = np.argsort(key, kind="stable")
    key_s = key[order]
    src_s, rel_s, w_s = src[order], rel[order], w[order]

    cnt = np.bincount(key_s, minlength=R * nch * 2).reshape(R, nch, 2)
    bh_arr = -(-cnt.max(axis=0) // P)          # [nch, 2] blocks
    bh_arr[:, 0] = np.maximum(bh_arr[:, 0], 1)
    bh = tuple((int(a), int(b)) for a, b in bh_arr)

    TB, sbo, call_specs, group_spans = layout_blocks(bh)

    # column offset (in the 128-edge-wide stream) for each (chunk, half)
    colbase = np.zeros((nch, 2), np.int64)
    for ci in range(nch):
        colbase[ci, 0] = sbo[ci][0]
        colbase[ci, 1] = sbo[ci][1]

    starts = np.searchsorted(key_s, np.arange(R * nch * 2 + 1))
    pos = np.arange(E) - starts[key_s]
    ch_s = (key_s // 2) % nch
    hf_s = key_s % 2
    co_s = key_s // (2 * nch)
    col = colbase[ch_s, hf_s] * P + pos         # position in per-core stream

    L_stream = TB * P
    idxs = np.zeros((R, L_stream), np.int32)    # gather idx (half-relative)
    rels = np.zeros((R, L_stream), np.float32)
    ws = np.zeros((R, L_stream), np.float32)
    idxs[co_s, col] = src_s - hf_s * HALF
    rels[co_s, col] = rel_s
    ws[co_s, col] = w_s
    assert idxs.max() < HALF and idxs.min() >= 0

    # gather idx wrapping: idx i -> partition i%16, column i//16, replicated
    # to all 8 16-partition groups
    idx16 = idxs.reshape(R, TB * 8, 16).transpose(0, 2, 1)   # [R, 16, TB*8]
    idx_wrap = np.ascontiguousarray(
        np.tile(idx16, (1, 8, 1)).astype(np.int16))          # [R, 128, TB*8]
    # rel/w: edge (tb, p) -> [128, TB]
    relsT = np.ascontiguousarray(
        rels.reshape(R, TB, P).transpose(0, 2, 1))
    wsT = np.ascontiguousarray(ws.reshape(R, TB, P).transpose(0, 2, 1))

    x_pad = np.zeros((n_pad2, D), np.float32)
    x_pad[:N] = x.astype(np.float32)
    x_bf = x_pad.astype(ml_dtypes.bfloat16)

    W2 = (W.T.astype(np.float32) * gamma.astype(np.float32)[:, None])
    W2c = W2 - np.ones((D, 1), np.float32) @ (W2.sum(axis=0, keepdims=True)) / D
    W2c = W2c.astype(ml_dtypes.bfloat16)
    c = beta.astype(np.float32) @ W.T.astype(np.float32) + b.astype(np.float32)
    assert not np.any(c != 0.0), "nonzero LN beta / linear bias unsupported"

    iota = np.broadcast_to(np.arange(P, dtype=np.float32), (P, P))
    iota = np.ascontiguousarray(iota).astype(ml_dtypes.bfloat16)
    ident = np.eye(P, dtype=np.float32).astype(ml_dtypes.bfloat16)

    in_maps = []
    for r in range(R):
        lo = r * npc
        hi = min(N, lo + npc)
        dsr = np.zeros(nch * P, np.float32)
        dsr[:hi - lo] = indsc[lo:hi]
        dsct = np.ascontiguousarray(dsr.reshape(nch, P).T)
        xres = np.zeros((nch * P, D), np.float32)
        xres[:hi - lo] = x_pad[lo:hi]
        in_maps.append({
            "xp": (np.ascontiguousarray(x_bf[r * rows_pc:(r + 1) * rows_pc])
                   if shard_p1 else x_bf),
            "xres": xres,
            "w2c": W2c,
            "iota": iota,
            "ident": ident,
            "gidx": idx_wrap[r],
            "rels": relsT[r],
            "ws": wsT[r],
            "dsct": dsct,
        })
    geom = dict(n_pad2=n_pad2, nch=nch, bh=bh, npc=npc, N=N, R=R, TB=TB)
    return in_maps, geom


_PROGRAM_CACHE = {}


def kernel(x, gamma, beta, W, b, edge_index, num_nodes, edge_weight,
           dst_scale, n_cores=8, _collect=None):
    x = np.asarray(x)
    N = x.shape[0]
    in_maps, geom = prepare_inputs(
        np.asarray(x), np.asarray(gamma), np.asarray(beta), np.asarray(W),
        np.asarray(b), np.asarray(edge_index), np.asarray(edge_weight),
        np.asarray(dst_scale), n_cores)

    key = (geom["n_pad2"], geom["nch"], geom["bh"], SHARD_P1)
    nc = _PROGRAM_CACHE.get(key)
    if nc is None:
        nc = build_program(geom["n_pad2"], geom["nch"], geom["bh"],
                           shard_p1=SHARD_P1, shared_m=SHARD_P1)
        nc.finalize()
        _PROGRAM_CACHE[key] = nc

    res = run_bass_kernel_spmd(nc, in_maps, list(range(n_cores)),
                               **(_collect.pop("kwargs") if _collect else {}))
    if _collect is not None:
        _collect["res"] = res

    y = np.empty((N, D), np.float32)
    npc = geom["npc"]
    for r in range(geom["R"]):
        lo = r * npc
        hi = min(N, lo + npc)
        y[lo:hi] = res.results[r]["y"][:hi - lo]
    return y


# revision 45
# speedup vs baseline: 4.4039x; 1.0720x over previous
"""Trainium2 Bass kernel for nn_DiWeightedGCNLayer (8-core SPMD), v2.

Math (per reference):
    h   = LayerNorm(x) * gamma + beta        (beta=0, b=0 here)
    m   = h @ W.T + b
    msg = m[src] * w
    out = segment_sum(msg, dst) / max(segment_sum(w, dst), 1) * dst_scale
    y   = x + gelu(out)

v2 design (vs the indirect-DMA baseline):
  Phase 1 (sharded): each core computes m for its 1/8 of node rows.
    LN's mean-subtraction is folded into the weight matrix on the host
    (W2c = W2 - ones @ colsum(W2)/D), so per 128-row tile we do:
    PE-transpose(x_bf16) -> matmul(x_T, W2c) -> scale rows by
    rstd = rsqrt(var+eps) (Act engine) -> m_part (bf16, HBM).
    An AllGather collective assembles the full m on every core.
  Phase 2: edges sorted by (dst chunk, src-half). Messages are fetched
    with batched gpsimd.dma_gather (hundreds of 256B rows per call,
    994ns fixed cost amortized) instead of one indirect DMA per 128
    edges. Scatter-add per 128-dst-node chunk stays the one-hot PE
    matmul (oh = (iota==rel)*w built by DVE).
    int16 gather indices force a split of m rows at 32768: each chunk's
    edges are grouped into src<32768 and src>=32768 blocks, gathered by
    two calls with different base row offsets.
"""

import contextlib
import numpy as np
import ml_dtypes

import concourse.bass as bass
import concourse.bacc as bacc
import concourse.tile as tile
import concourse.mybir as mybir
from concourse.bass_utils import run_bass_kernel_spmd

F32 = mybir.dt.float32
BF16 = mybir.dt.bfloat16
I32 = mybir.dt.int32
I16 = mybir.dt.int16
AF = mybir.ActivationFunctionType
OP = mybir.AluOpType

D = 128
P = 128
LN_EPS = 1e-5
R = 8
HALF = 32768
GC = 8  # chunks per gather group
SHARD_P1 = True  # ship sharded phase 1 + AllGather


def layout_blocks(bh, gc=GC):
    """Global block-column layout: group-major, half-major inside a group.
    Returns (TB, sbo, call_specs, group_spans):
      sbo[ci] = (col of ci's first h0 block, col of ci's first h1 block)
      call_specs[g] = (tb0, nb0, tb1, nb1)  (h0/h1 stream offsets+lengths)
      group_spans[g] = (ci_start, ci_end, tb_base, nb_total)
    """
    nch = len(bh)
    tb = 0
    sbo = {}
    call_specs = []
    group_spans = []
    for cs in range(0, nch, gc):
        g = list(range(cs, min(nch, cs + gc)))
        nb0 = sum(bh[ci][0] for ci in g)
        nb1 = sum(bh[ci][1] for ci in g)
        cur0, cur1 = tb, tb + nb0
        for ci in g:
            sbo[ci] = (cur0, cur1)
            cur0 += bh[ci][0]
            cur1 += bh[ci][1]
        call_specs.append((tb, nb0, tb + nb0, nb1))
        group_spans.append((cs, min(nch, cs + gc), tb, nb0 + nb1))
        tb += nb0 + nb1
    return tb, sbo, call_specs, group_spans


def build_program(n_pad2, nch, bh, shard_p1=True, loop_n=1, g_tiles=None,
                  n_swdge=4, msg_bufs=3, oh_bufs=12, skip_gather=False,
                  skip_p1=False, cc_emu=False, shared_m=False,
                  skip_compute=False, skip_mm=False, oh_pool_frac=0.0,
                  unroll=False):
    """One-core SPMD program. bh: tuple of (h0_blocks, h1_blocks) per chunk
    (identical across cores = max over cores)."""
    rows_pc = n_pad2 // R          # node rows computed per core in phase 1
    tpc = rows_pc // P             # tiles per core
    nt_p1 = tpc if shard_p1 else n_pad2 // P
    if g_tiles is None:
        g_tiles = 7 if shard_p1 else 14
    assert nt_p1 % g_tiles == 0
    TB, sbo, call_specs, group_spans = layout_blocks(bh)

    nc = bacc.Bacc(num_swdge_queues=n_swdge, num_devices=R)

    xp_rows = rows_pc if shard_p1 else n_pad2
    xp_ext = nc.declare_dram_parameter("xp", [xp_rows, D], BF16, isOutput=False)
    xres_ext = nc.declare_dram_parameter("xres", [nch * P, D], F32,
                                         isOutput=False)
    w2c_ext = nc.declare_dram_parameter("w2c", [D, D], BF16, isOutput=False)
    iota_ext = nc.declare_dram_parameter("iota", [P, P], BF16, isOutput=False)
    ident_ext = nc.declare_dram_parameter("ident", [P, P], BF16, isOutput=False)
    idx_ext = nc.declare_dram_parameter("gidx", [P, TB * 8], I16, isOutput=False)
    rel_ext = nc.declare_dram_parameter("rels", [P, TB], F32, isOutput=False)
    w_ext = nc.declare_dram_parameter("ws", [P, TB], F32, isOutput=False)
    dsc_ext = nc.declare_dram_parameter("dsct", [P, nch], F32, isOutput=False)
    y_ext = nc.declare_dram_parameter("y", [nch * P, D], F32, isOutput=True)

    m_full = nc.dram_tensor("m_full", [n_pad2, D], BF16,
                            addr_space="Shared" if shared_m else "Local")
    if shard_p1:
        m_part = nc.dram_tensor("m_part", [rows_pc, D], BF16)
    if cc_emu and not shared_m:
        mrest_ext = nc.declare_dram_parameter(
            "m_rest", [n_pad2 - rows_pc, D], BF16, isOutput=False)

    with tile.TileContext(nc) as tc:
        with (
            tc.tile_pool(name="const", bufs=1) as const,
            tc.tile_pool(name="xp", bufs=3) as xpp,
            tc.tile_pool(name="stats", bufs=4) as sp,
            tc.tile_pool(name="small", bufs=6) as smp,
            tc.tile_pool(name="xts", bufs=3) as xtsp,
            tc.tile_pool(name="mp", bufs=3) as mp,
            tc.tile_pool(name="msg", bufs=msg_bufs) as msgp,
            tc.tile_pool(name="oh", bufs=oh_bufs) as ohp,
            tc.tile_pool(name="ep", bufs=2) as epp,
            tc.tile_pool(name="ps_t", bufs=2, space="PSUM") as ps_t,
            tc.tile_pool(name="ps_m", bufs=2, space="PSUM") as ps_m,
            tc.tile_pool(name="ps_o", bufs=3, space="PSUM") as ps_o,
        ):
            # --- constants (outside the benchmark loop) ---
            w2c_t = const.tile([D, D], BF16)
            nc.sync.dma_start(out=w2c_t[:], in_=w2c_ext[:, :])
            iota_t = const.tile([P, P], BF16)
            nc.sync.dma_start(out=iota_t[:], in_=iota_ext[:, :])
            ident = const.tile([P, P], BF16)
            nc.sync.dma_start(out=ident[:], in_=ident_ext[:, :])
            eps_t = const.tile([P, 1], F32)
            nc.vector.memset(eps_t[:], LN_EPS)
            dsc_t = const.tile([P, nch], F32)
            nc.sync.dma_start(out=dsc_t[:], in_=dsc_ext[:, :])
            idx_t = const.tile([P, TB * 8], I16)
            nc.sync.dma_start(out=idx_t[:], in_=idx_ext[:, :])
            rel_t = const.tile([P, TB], F32)
            nc.sync.dma_start(out=rel_t[:], in_=rel_ext[:, :])
            w_t = const.tile([P, TB], F32)
            nc.sync.dma_start(out=w_t[:], in_=w_ext[:, :])

            loop_ctx = (tc.For_i(0, loop_n, 1) if loop_n > 1 and not unroll
                        else contextlib.nullcontext())
            for _unroll_i in range(loop_n if unroll else 1):
              with loop_ctx:
                # --- phase 1: m = rstd * (x_bf16 @ W2c), bf16 to HBM ---
                # Node rows permuted inside each supertile of 128*G rows:
                # row (t, p, j) = t*128G + p*G + j -> partition p, slot j.
                # One contiguous G*256B descriptor per partition per DMA.
                m_dst_dram = m_part if shard_p1 else m_full
                G = g_tiles
                for t0 in range(0, 0 if skip_p1 else nt_p1, G):
                    g_n = min(G, nt_p1 - t0)
                    xt4 = xpp.tile([P, G, D], BF16)
                    x_src = xp_ext[t0 * P:(t0 + g_n) * P, :].rearrange(
                        "(p j) d -> p j d", p=P)
                    nc.sync.dma_start(out=xt4[:, :g_n, :], in_=x_src)
                    m4 = mp.tile([P, G, D], BF16)
                    for j in range(g_n):
                        xt = xt4[:, j, :]
                        st = sp.tile([P, 6], F32)
                        nc.vector.bn_stats(out=st[:], in_=xt)
                        mv = sp.tile([P, 2], F32)
                        nc.vector.bn_aggr(out=mv[:], in_=st[:])
                        sd = smp.tile([P, 1], F32)
                        nc.scalar.activation(out=sd[:], in_=mv[:, 1:2],
                                             func=AF.Sqrt, bias=eps_t[:, :],
                                             scale=1.0)
                        rstd = smp.tile([P, 1], F32)
                        nc.vector.reciprocal(out=rstd[:], in_=sd[:])
                        xt_ps = ps_t.tile([P, D], BF16)
                        nc.tensor.transpose(out=xt_ps[:], in_=xt,
                                            identity=ident[:])
                        xts = xtsp.tile([P, D], BF16)
                        nc.scalar.copy(out=xts[:], in_=xt_ps[:])
                        m_ps = ps_m.tile([P, D], F32)
                        nc.tensor.matmul(out=m_ps[:], lhsT=xts[:], rhs=w2c_t[:],
                                         start=True, stop=True)
                        nc.vector.tensor_scalar(out=m4[:, j, :], in0=m_ps[:],
                                                scalar1=rstd[:], scalar2=None,
                                                op0=OP.mult)
                    m_dst = m_dst_dram[t0 * P:(t0 + g_n) * P, :].rearrange(
                        "(p j) d -> p j d", p=P)
                    nc.sync.dma_start(out=m_dst, in_=m4[:, :g_n, :])

                if shard_p1 and cc_emu:
                    # benchmark stand-in for AllGather (loop-unsafe on HW):
                    # local HBM copies with the same written byte count
                    nc.sync.dma_start(out=m_full[0:rows_pc, :],
                                      in_=m_part[:, :])
                    if not shared_m:
                        nc.sync.dma_start(out=m_full[rows_pc:n_pad2, :],
                                          in_=mrest_ext[:, :])
                elif shard_p1:
                    nc.gpsimd.collective_compute(
                        "AllGather", OP.bypass,
                        replica_groups=[list(range(R))],
                        ins=[m_part[:, :]],
                        outs=[m_full[:, :]],
                    )

                # --- phase 2: batched gather + one-hot scatter matmul ---
                qload = [0] * n_swdge  # greedy least-loaded queue assignment
                nreg_cache = {}
                for gidx, (cs, ce, tb_base, nb_tot) in enumerate(group_spans):
                    tb0, nb0, tb1, nb1 = call_specs[gidx]
                    msg = msgp.tile([P, nb_tot, D], BF16)
                    # SWDGE ring caps one call at 1024 descriptors (8 blocks)
                    CB = 8
                    for h, (tbh, nbh) in enumerate(((tb0, nb0), (tb1, nb1))):
                        base = h * HALF
                        for s0 in range(0, nbh, CB):
                            sn = min(CB, nbh - s0)
                            L = sn * P
                            col0 = tbh - tb_base + s0
                            if skip_gather:
                                nc.vector.memset(msg[:, col0:col0 + sn, :],
                                                 0.25)
                                continue
                            if L not in nreg_cache:
                                nreg_cache[L] = nc.gpsimd.to_reg(L)
                            qi = min(range(n_swdge), key=lambda q: qload[q])
                            qload[qi] += L
                            nc.gpsimd.dma_gather(
                                msg[:, col0:col0 + sn, :],
                                m_full[base:n_pad2, :],
                                idx_t[:, (tbh + s0) * 8:(tbh + s0 + sn) * 8],
                                num_idxs=L,
                                num_idxs_reg=nreg_cache[L],
                                elem_size=D,
                                queue_num=qi,
                            )
                    ng = ce - cs
                    xr = epp.tile([P, GC, D], F32, tag="xr")
                    xr_src = xres_ext[cs * P:ce * P, :].rearrange(
                        "(j p) d -> p j d", p=P)
                    nc.sync.dma_start(out=xr[:, :ng, :], in_=xr_src)
                    yt = epp.tile([P, GC, D], F32, tag="yt")
                    if skip_compute:
                        nc.vector.tensor_scalar(out=yt[:, 0, :],
                                                in0=msg[:, 0, :],
                                                scalar1=1.0, scalar2=None,
                                                op0=OP.mult)
                        y_dst = y_ext[cs * P:ce * P, :].rearrange(
                            "(j p) d -> p j d", p=P)
                        nc.sync.dma_start(out=y_dst, in_=yt[:, :ng, :])
                        continue
                    for ci in range(cs, ce):
                        c0, c1 = sbo[ci]
                        cols = ([c0 + b for b in range(bh[ci][0])]
                                + [c1 + b for b in range(bh[ci][1])])
                        out_ps = ps_o.tile([P, D], F32)
                        nb = len(cols)
                        for k, tb in enumerate(cols):
                            oh = ohp.tile([P, P], BF16)
                            oh_eng = (nc.gpsimd if (k % 100) < oh_pool_frac * 100
                                      else nc.vector)
                            oh_eng.tensor_scalar(out=oh[:], in0=iota_t[:],
                                                 scalar1=rel_t[:, tb:tb + 1],
                                                 scalar2=w_t[:, tb:tb + 1],
                                                 op0=OP.is_equal,
                                                 op1=OP.mult)
                            if skip_mm:
                                continue
                            mcol = tb - tb_base
                            nc.tensor.matmul(out=out_ps[:], lhsT=oh[:],
                                             rhs=msg[:, mcol, :],
                                             start=(k == 0), stop=(k == nb - 1))

                        # dsc_t holds host-precomputed dst_scale/max(deg,1)
                        sc = epp.tile([P, D], F32, tag="sc")
                        nc.vector.tensor_scalar(out=sc[:], in0=out_ps[:],
                                                scalar1=dsc_t[:, ci:ci + 1],
                                                scalar2=None, op0=OP.mult)
                        g = epp.tile([P, D], F32, tag="g")
                        nc.scalar.activation(out=g[:], in_=sc[:], func=AF.Gelu)
                        nc.vector.tensor_add(out=yt[:, ci - cs, :], in0=g[:],
                                             in1=xr[:, ci - cs, :])
                    y_dst = y_ext[cs * P:ce * P, :].rearrange(
                        "(j p) d -> p j d", p=P)
                    nc.sync.dma_start(out=y_dst, in_=yt[:, :ng, :])

    return nc


def prepare_inputs(x, gamma, beta, W, b, edge_index, edge_weight, dst_scale,
                   n_cores, shard_p1=None):
    if shard_p1 is None:
        shard_p1 = SHARD_P1
    """Host-side prep: sort edges by (dst-chunk, src-half), build gather
    index/rel/weight streams; fold LN gamma + mean-subtraction into W2c."""
    N = x.shape[0]
    assert n_cores == R
    npc = N // R                       # dst ownership per core (6250)
    nch = (npc + P - 1) // P           # 49
    n_pad2 = ((N + R * P - 1) // (R * P)) * R * P  # 50176
    rows_pc = n_pad2 // R              # 6272

    src = np.ascontiguousarray(edge_index[0]).astype(np.int64)
    dst = np.ascontiguousarray(edge_index[1]).astype(np.int64)
    w = edge_weight.astype(np.float32)
    E = src.shape[0]

    deg = np.bincount(dst, weights=w.astype(np.float64), minlength=N)
    indsc = (dst_scale.astype(np.float64)
             / np.maximum(deg, 1.0)).astype(np.float32)

    core_id = np.minimum(dst // npc, R - 1)
    local = dst - core_id * npc
    chunk_id = local // P
    rel = (local - chunk_id * P).astype(np.float32)
    half = (src >= HALF).astype(np.int64)
    key = (core_id * nch + chunk_id) * 2 + half
    order = np.argsort(key, kind="stable")
    key_s = key[order]
    src_s, rel_s, w_s = src[order], rel[order], w[order]

    cnt = np.bincount(key_s, minlength=R * nch * 2).reshape(R, nch, 2)
    bh_arr = -(-cnt.max(axis=0) // P)          # [nch, 2] blocks
    bh_arr[:, 0] = np.maximum(bh_arr[:, 0], 1)
    bh = tuple((int(a), int(b)) for a, b in bh_arr)

    TB, sbo, call_specs, group_spans = layout_blocks(bh)

    # column offset (in the 128-edge-wide stream) for each (chunk, half)
    colbase = np.zeros((nch, 2), np.int64)
    for ci in range(nch):
        colbase[ci, 0] = sbo[ci][0]
        colbase[ci, 1] = sbo[ci][1]

    starts = np.searchsorted(key_s, np.arange(R * nch * 2 + 1))
    pos = np.arange(E) - starts[key_s]
    ch_s = (key_s // 2) % nch
    hf_s = key_s % 2
    co_s = key_s // (2 * nch)
    col = colbase[ch_s, hf_s] * P + pos         # position in per-core stream

    L_stream = TB * P
    idxs = np.zeros((R, L_stream), np.int32)    # gather idx (half-relative)
    rels = np.zeros((R, L_stream), np.float32)
    ws = np.zeros((R, L_stream), np.float32)
    idxs[co_s, col] = src_s - hf_s * HALF
    rels[co_s, col] = rel_s
    ws[co_s, col] = w_s
    assert idxs.max() < HALF and idxs.min() >= 0

    # gather idx wrapping: idx i -> partition i%16, column i//16, replicated
    # to all 8 16-partition groups
    idx16 = idxs.reshape(R, TB * 8, 16).transpose(0, 2, 1)   # [R, 16, TB*8]
    idx_wrap = np.ascontiguousarray(
        np.tile(idx16, (1, 8, 1)).astype(np.int16))          # [R, 128, TB*8]
    # rel/w: edge (tb, p) -> [128, TB]
    relsT = np.ascontiguousarray(
        rels.reshape(R, TB, P).transpose(0, 2, 1))
    wsT = np.ascontiguousarray(ws.reshape(R, TB, P).transpose(0, 2, 1))

    x_pad = np.zeros((n_pad2, D), np.float32)
    x_pad[:N] = x.astype(np.float32)
    x_bf = x_pad.astype(ml_dtypes.bfloat16)

    W2 = (W.T.astype(np.float32) * gamma.astype(np.float32)[:, None])
    W2c = W2 - np.ones((D, 1), np.float32) @ (W2.sum(axis=0, keepdims=True)) / D
    W2c = W2c.astype(ml_dtypes.bfloat16)
    c = beta.astype(np.float32) @ W.T.astype(np.float32) + b.astype(np.float32)
    assert not np.any(c != 0.0), "nonzero LN beta / linear bias unsupported"

    iota = np.broadcast_to(np.arange(P, dtype=np.float32), (P, P))
    iota = np.ascontiguousarray(iota).astype(ml_dtypes.bfloat16)
    ident = np.eye(P, dtype=np.float32).astype(ml_dtypes.bfloat16)

    in_maps = []
    for r in range(R):
        lo = r * npc
        hi = min(N, lo + npc)
        dsr = np.zeros(nch * P, np.float32)
        dsr[:hi - lo] = indsc[lo:hi]
        dsct = np.ascontiguousarray(dsr.reshape(nch, P).T)
        xres = np.zeros((nch * P, D), np.float32)
        xres[:hi - lo] = x_pad[lo:hi]
        in_maps.append({
            "xp": (np.ascontiguousarray(x_bf[r * rows_pc:(r + 1) * rows_pc])
                   if shard_p1 else x_bf),
            "xres": xres,
            "w2c": W2c,
            "iota": iota,
            "ident": ident,
            "gidx": idx_wrap[r],
            "rels": relsT[r],
            "ws": wsT[r],
            "dsct": dsct,
        })
    geom = dict(n_pad2=n_pad2, nch=nch, bh=bh, npc=npc, N=N, R=R, TB=TB)
    return in_maps, geom


_PROGRAM_CACHE = {}


def kernel(x, gamma, beta, W, b, edge_index, num_nodes, edge_weight,
           dst_scale, n_cores=8, _collect=None):
    x = np.asarray(x)
    N = x.shape[0]
    in_maps, geom = prepare_inputs(
        np.asarray(x), np.asarray(gamma), np.asarray(beta), np.asarray(W),
        np.asarray(b), np.asarray(edge_index), np.asarray(edge_weight),
        np.asarray(dst_scale), n_cores)

    key = (geom["n_pad2"], geom["nch"], geom["bh"], SHARD_P1)
    nc = _PROGRAM_CACHE.get(key)
    if nc is None:
        nc = build_program(geom["n_pad2"], geom["nch"], geom["bh"],
                           shard_p1=SHARD_P1, shared_m=SHARD_P1)
        nc.finalize()
        _PROGRAM_CACHE[key] = nc

    res = run_bass_kernel_spmd(nc, in_maps, list(range(n_cores)),
                               **(_collect.pop("kwargs") if _collect else {}))
    if _collect is not None:
        _collect["res"] = res

    y = np.empty((N, D), np.float32)
    npc = geom["npc"]
    for r in range(geom["R"]):
        lo = r * npc
        hi = min(N, lo + npc)
        y[lo:hi] = res.results[r]["y"][:hi - lo]
    return y
